# revision 11
# baseline (speedup 1.0000x reference)
"""Trainium2 Bass kernel for nn_L2Accuracy (segment_reduce).

Computes, for pred/target [B=32, N=200000, D=3] and ragged segment
boundaries `indices` [B, 9]:
    err[b, n] = ||pred[b,n] - target[b,n]||_2
    per-(batch, segment) sums of err  (device, 8 NeuronCores)
    segment means + per-type means    (host, O(B*G) scalars)

Device strategy (data-parallel over batch, 4 batches/core):
  - Each (batch, range) is a contiguous vertex run [a, b).  Pieces of a
    range are loaded as [P, F] SBUF tiles (F divisible by 3 so vertices
    never straddle partitions), several ranges column-blocked into one
    supertile so the elementwise work runs as a few big ops:
      gpsimd : diff = pred - target
      scalar : diff = diff^2            (in-place)
      vector : sv   = sum over D=3      (3D-view reduce, axis X)
      scalar : sv   = sqrt(sv), accum_out -> per-partition range partial
      tensor : ones^T @ acc  -> per-piece sums [1, ncols]
  - Host folds piece sums -> range sums -> segment sums (incl. the
    reference's tail-aliasing into the next batch's segment 0), then
    segment means -> per-type means.
"""

import os
import sys

sys.path.insert(0, "/opt/trn_rl_repo")

import numpy as np

B, N, D = 32, 200000, 3
G, T = 8, 5
NCORES = 8
BPC = B // NCORES          # batches per core (fast path)
EPB = N * D                # elements per batch
FMAX_V = 1600              # max vertices per partition in one piece
STILE_V = 1600             # max vertices per partition in one supertile

_prog_cache = {}


# ---------------------------------------------------------------- host schedule


def _ranges_from_bnd(bnd):
    """9 contiguous vertex ranges partitioning [0, N) for one batch.

    Range r in 0..7 holds vertices with sid == r per the reference's
    searchsorted(bnd[1:], pos, 'right'); range 8 is the tail [bnd[8], N)
    whose vertices alias into the next batch's segment 0.
    """
    starts = [0] + [int(bnd[j]) for j in range(1, G + 1)]
    ends = [int(bnd[j]) for j in range(1, G + 1)] + [N]
    return [(s, max(0, e - s)) for s, e in zip(starts, ends)]


def _pieces_for_range(voff, vcnt):
    """Cover vcnt vertices from voff with [P, F] rects, 3 | F, P <= 128."""
    if vcnt == 0:
        return []
    for P in range(128, 63, -1):
        if vcnt % P == 0 and vcnt // P <= FMAX_V:
            return [(voff, P, 3 * (vcnt // P))]
    pieces, v, left = [], voff, vcnt
    while left > 0:
        P = min(128, left)
        fv = max(1, min(FMAX_V, left // P))
        pieces.append((v, P, 3 * fv))
        v += P * fv
        left -= P * fv
    return pieces


def _build_table(bnds):
    """Piece/supertile schedule for a list of per-batch boundary rows.

    Returns (supertiles, col_map, ncols):
      supertiles: list of (P, [(elem_off, F, col), ...])
      col_map:    col -> (batch_local, range_idx)
    """
    pieces = []          # (batch_local, range_idx, elem_off, P, F)
    for bl, bnd in enumerate(bnds):
        for r, (vs, vc) in enumerate(_ranges_from_bnd(bnd)):
            for (v0, P, F) in _pieces_for_range(vs, vc):
                pieces.append((bl, r, bl * EPB + 3 * v0, P, F))

    supertiles, col_map = [], []
    cur_p, cur_list, cur_fv = None, [], 0
    for (bl, r, eoff, P, F) in pieces:
        col = len(col_map)
        col_map.append((bl, r))
        if cur_p != P or cur_fv + F // 3 > STILE_V:
            if cur_list:
                supertiles.append((cur_p, cur_list))
            cur_p, cur_list, cur_fv = P, [], 0
        cur_list.append((eoff, F, col))
        cur_fv += F // 3
    if cur_list:
        supertiles.append((cur_p, cur_list))
    return supertiles, col_map, len(col_map)


# ---------------------------------------------------- fastest SWDGE-cast path
#
# HW findings (perfetto, axon trn2): HWDGE (nc.sync/nc.scalar) queues are
# served by only 5 SDMA engines (~134 GB/s); SWDGE (nc.gpsimd) sprays
# descriptors across all 16 engines.  With all 16 engines the SBUF-write
# port mux (~13 GB/s/engine) binds for f32, so casting f32->bf16 during
# the SWDGE DMA halves the write side and lets the HBM read side run at
# ~26.6 GB/s/engine (~400 GB/s aggregate).  Loads: 2 two-batch supertiles
# [128, 9375] per tensor per core; bf16 compute, f32 accumulation.
#
# Requires: all batches share one boundary vector whose entries divide by
# bs = gcd(3125, boundaries) >= 125 (each partition row holds 3125
# vertices; batch = 64 rows exactly), J = 3125/bs block sums per row.


def _fast3_ok(bnds):
    """fast3 needs equal per-batch boundaries, all multiples of 6250."""
    if not all((bnds[i] == bnds[0]).all() for i in range(1, len(bnds))):
        return False
    return all(int(v) % 6250 == 0 for v in bnds[0].tolist())


def _build_program_fast3():
    """One [128, 18750] bf16 tile pair per core (4 batches, row = 6250
    vertices, batch = 32 rows).  Loads are SWDGE f32->bf16 casts split
    into interleaved pred/target column-quarter DMAs so the compute
    chain (vector sub -> scalar square -> vector D-reduce -> scalar
    sqrt+accum) pipelines behind the DMA stream; eb[:, 0] collects
    per-row err sums for host assembly."""
    import concourse.bacc as bacc
    import concourse.mybir as mybir
    from concourse.tile import TileContext

    f32 = mybir.dt.float32
    bf16 = mybir.dt.bfloat16
    Act = mybir.ActivationFunctionType

    W = 18750
    # SWDGE bf16 stages (col ranges, multiples of 3), decreasing so the
    # last stage (whose compute is the post-DMA tail) is small; the final
    # HW columns go via the independent HWDGE path (sync, f32, engines
    # E64-68) so the tail does not wait on the slowest SWDGE engine
    HW = 1125
    qw = [5625, 5625, 4500, 1875]
    qcols = []
    a = 0
    for w in qw:
        qcols.append((a, a + w))
        a += w
    assert a == W - HW
    # compute chunks: halves of the wide stages
    ccols = []
    for i, (a, b) in enumerate(qcols):
        if b - a > 2000:
            m = a + ((b - a) // 2 // 3) * 3
            ccols.append((a, m))
            ccols.append((m, b))
        else:
            ccols.append((a, b))
    ncc = len(ccols) + 1

    nc = bacc.Bacc(
        "TRN2", target_bir_lowering=False, debug=False, num_devices=NCORES
    )
    pred_t = nc.dram_tensor("pred", [BPC * EPB], f32, kind="ExternalInput").ap()
    targ_t = nc.dram_tensor("target", [BPC * EPB], f32, kind="ExternalInput").ap()
    out_t = nc.dram_tensor("out", [128, ncc], f32, kind="ExternalOutput").ap()

    with TileContext(nc) as tc:
        with (
            tc.tile_pool(name="io", bufs=1) as io_pool,
            tc.tile_pool(name="work", bufs=1) as w_pool,
            tc.tile_pool(name="stat", bufs=1) as s_pool,
        ):
            eb = s_pool.tile([128, ncc], f32)
            nc.gpsimd.memset(eb[:], 0.0)
            tp = io_pool.tile([128, W], bf16, tag="tp")
            tt = io_pool.tile([128, W], bf16, tag="tt")
            diff = w_pool.tile([128, W], bf16, tag="diff")
            t1 = w_pool.tile([128, W // 3], bf16, tag="t1")
            sv = w_pool.tile([128, W // 3], bf16, tag="sv")

            tpf = w_pool.tile([128, HW], f32, tag="tpf")
            ttf = w_pool.tile([128, HW], f32, tag="ttf")
            svf = w_pool.tile([128, HW // 3], f32, tag="svf")

            psrc = pred_t.rearrange("(p f) -> p f", p=128)
            tsrc = targ_t.rearrange("(p f) -> p f", p=128)
            nc.sync.dma_start(tpf[:], psrc[:, W - HW : W])
            nc.sync.dma_start(ttf[:], tsrc[:, W - HW : W])
            for (a, b) in qcols:
                nc.gpsimd.dma_start(tp[:, a:b], psrc[:, a:b])
                nc.gpsimd.dma_start(tt[:, a:b], tsrc[:, a:b])

            for ci, (a, b) in enumerate(ccols):
                nc.vector.tensor_tensor(
                    diff[:, a:b],
                    tp[:, a:b],
                    tt[:, a:b],
                    mybir.AluOpType.subtract,
                )
                nc.scalar.activation(diff[:, a:b], diff[:, a:b], Act.Square)
                va, vb = a // 3, b // 3
                sq3 = diff[:, a:b].rearrange("p (v d) -> p v d", d=3)
                nc.vector.tensor_tensor(
                    t1[:, va:vb], sq3[:, :, 0], sq3[:, :, 1], mybir.AluOpType.add
                )
                nc.vector.tensor_tensor(
                    sv[:, va:vb], t1[:, va:vb], sq3[:, :, 2], mybir.AluOpType.add
                )
                nc.scalar.activation(
                    sv[:, va:vb],
                    sv[:, va:vb],
                    Act.Sqrt,
                    accum_out=eb[:, ci : ci + 1],
                )

            # HWDGE f32 tail slice
            nc.vector.tensor_tensor(
                tpf[:], tpf[:], ttf[:], mybir.AluOpType.subtract
            )
            nc.scalar.activation(tpf[:], tpf[:], Act.Square)
            sq3f = tpf[:].rearrange("p (v d) -> p v d", d=3)
            nc.vector.tensor_tensor(
                svf[:], sq3f[:, :, 0], sq3f[:, :, 1], mybir.AluOpType.add
            )
            nc.vector.tensor_tensor(
                svf[:], svf[:], sq3f[:, :, 2], mybir.AluOpType.add
            )
            nc.scalar.activation(
                svf[:], svf[:], Act.Sqrt, accum_out=eb[:, ncc - 1 : ncc]
            )
            nc.sync.dma_start(out_t, eb[:])

    nc.compile()
    return nc


def _fast3_host_assemble(core_outs, bnd0):
    """core_outs: per-core [128, 8] chunk sums (one col per compute
    chunk) -> piece_sums [B, G+1].

    Row p of a core holds vertices [p*6250, (p+1)*6250) of the core's
    4-batch blob; batch bl = rows 32*bl .. +32."""
    piece_sums = np.zeros((B, G + 1), dtype=np.float64)
    edges = [0] + [int(bnd0[j]) // 6250 for j in range(1, G + 1)] + [32]
    for c, out in enumerate(core_outs):
        rows = out.sum(axis=1, dtype=np.float64).reshape(128)
        for bl in range(BPC):
            flat = rows[32 * bl : 32 * bl + 32]
            csum = np.concatenate([[0.0], np.cumsum(flat, dtype=np.float64)])
            for r in range(G + 1):
                piece_sums[c * BPC + bl, r] = csum[edges[r + 1]] - csum[edges[r]]
    return piece_sums


def _fast2_bs(bnds):
    import math

    if not all((bnds[i] == bnds[0]).all() for i in range(1, len(bnds))):
        return None
    bs = 3125
    for v in bnds[0].tolist():
        bs = math.gcd(bs, int(v))
    return bs if bs >= 125 else None


def _build_program_fast2(bs):
    import concourse.bacc as bacc
    import concourse.mybir as mybir
    from concourse.tile import TileContext

    f32 = mybir.dt.float32
    bf16 = mybir.dt.bfloat16
    Act = mybir.ActivationFunctionType
    J = 3125 // bs
    ncols = 2 * J

    nc = bacc.Bacc(
        "TRN2", target_bir_lowering=False, debug=False, num_devices=NCORES
    )
    pred_t = nc.dram_tensor("pred", [BPC * EPB], f32, kind="ExternalInput").ap()
    targ_t = nc.dram_tensor("target", [BPC * EPB], f32, kind="ExternalInput").ap()
    out_t = nc.dram_tensor("out", [128, ncols], f32, kind="ExternalOutput").ap()

    with TileContext(nc) as tc:
        with (
            tc.tile_pool(name="io", bufs=4) as io_pool,
            tc.tile_pool(name="work", bufs=2) as w_pool,
            tc.tile_pool(name="stat", bufs=1) as s_pool,
        ):
            eb = s_pool.tile([128, ncols], f32)
            nc.gpsimd.memset(eb[:], 0.0)
            for s in range(2):
                # supertile s = batches (2s, 2s+1): elems [2s*EPB, (2s+2)*EPB)
                tp = io_pool.tile([128, 9375], bf16, tag="tp")
                tt = io_pool.tile([128, 9375], bf16, tag="tt")
                src = pred_t[2 * s * EPB : (2 * s + 2) * EPB].rearrange(
                    "(p f) -> p f", p=128
                )
                nc.gpsimd.dma_start(tp[:], src)
                src = targ_t[2 * s * EPB : (2 * s + 2) * EPB].rearrange(
                    "(p f) -> p f", p=128
                )
                nc.gpsimd.dma_start(tt[:], src)
                diff = w_pool.tile([128, 9375], bf16, tag="diff")
                nc.vector.tensor_tensor(
                    diff[:], tp[:], tt[:], mybir.AluOpType.subtract
                )
                nc.scalar.activation(diff[:], diff[:], Act.Square)
                sv = w_pool.tile([128, 3125], f32, tag="sv")
                nc.vector.tensor_reduce(
                    sv[:],
                    diff[:].rearrange("p (v d) -> p v d", d=3),
                    axis=mybir.AxisListType.X,
                    op=mybir.AluOpType.add,
                )
                if J == 1:
                    nc.scalar.activation(
                        sv[:], sv[:], Act.Sqrt, accum_out=eb[:, s : s + 1]
                    )
                else:
                    nc.scalar.activation(sv[:], sv[:], Act.Sqrt)
                    nc.vector.tensor_reduce(
                        eb[:, s * J : (s + 1) * J],
                        sv[:].rearrange("p (j v) -> p j v", v=bs),
                        axis=mybir.AxisListType.X,
                        op=mybir.AluOpType.add,
                    )
            nc.sync.dma_start(out_t, eb[:])

    nc.compile()
    return nc


def _fast2_host_assemble(core_outs, bnd0, bs):
    """core_outs: per-core [128, 2J] block sums -> piece_sums [B, G+1].

    Batch bl of a core lives in supertile s = bl//2, rows 64*(bl%2)..+64;
    within a batch the flat block order is (row, j), block g covering
    vertices [g*bs, (g+1)*bs).
    """
    J = 3125 // bs
    nblk = 64 * J
    edges = [0] + [int(bnd0[j]) // bs for j in range(1, G + 1)] + [nblk]
    piece_sums = np.zeros((B, G + 1), dtype=np.float64)
    for c, out in enumerate(core_outs):
        for bl in range(BPC):
            s, half = divmod(bl, 2)
            flat = out[64 * half : 64 * half + 64, s * J : (s + 1) * J].reshape(-1)
            csum = np.concatenate([[0.0], np.cumsum(flat, dtype=np.float64)])
            for r in range(G + 1):
                piece_sums[c * BPC + bl, r] = csum[edges[r + 1]] - csum[edges[r]]
    return piece_sums


# ------------------------------------------------------- fast block-sum path
#
# When all batches share one boundary vector whose entries divide by a
# block size bs (bs | 800, bs >= 50), each batch is two [125, 2400]-elem
# half-tiles (one contiguous 9.6 KB run per partition -> ~125 DMA packets
# per 1.2 MB DMA instead of per-range shattering), and per-(row, block)
# err sums [125, 2*J2] per batch stream out for host reduceat assembly.


def _fast_bs(bnds):
    import math

    if not all((bnds[i] == bnds[0]).all() for i in range(1, len(bnds))):
        return None
    bs = 800
    for v in bnds[0].tolist():
        bs = math.gcd(bs, int(v))
    return bs if bs >= 50 else None


def _build_program_fast(bs):
    import concourse.bacc as bacc
    import concourse.mybir as mybir
    from concourse.tile import TileContext

    f32 = mybir.dt.float32
    Act = mybir.ActivationFunctionType
    J2 = 800 // bs  # blocks per half-row
    ncols = BPC * 2 * J2

    nc = bacc.Bacc(
        "TRN2", target_bir_lowering=False, debug=False, num_devices=NCORES
    )
    pred_t = nc.dram_tensor("pred", [BPC * EPB], f32, kind="ExternalInput").ap()
    targ_t = nc.dram_tensor("target", [BPC * EPB], f32, kind="ExternalInput").ap()
    out_t = nc.dram_tensor("out", [125, ncols], f32, kind="ExternalOutput").ap()

    with TileContext(nc) as tc:
        with (
            tc.tile_pool(name="io", bufs=4) as io_pool,
            tc.tile_pool(name="work", bufs=3) as w_pool,
            tc.tile_pool(name="stat", bufs=1) as s_pool,
        ):
            eb = s_pool.tile([125, ncols], f32)
            for b in range(BPC):
                for h in range(2):
                    # partition p holds elements [b*EPB + 4800p + 2400h, +2400)
                    tp = io_pool.tile([125, 2400], f32, tag="tp")
                    tt = io_pool.tile([125, 2400], f32, tag="tt")
                    src = pred_t[b * EPB : (b + 1) * EPB].rearrange(
                        "(p f) -> p f", p=125
                    )[:, 2400 * h : 2400 * h + 2400]
                    nc.sync.dma_start(tp[:], src)
                    src = targ_t[b * EPB : (b + 1) * EPB].rearrange(
                        "(p f) -> p f", p=125
                    )[:, 2400 * h : 2400 * h + 2400]
                    nc.sync.dma_start(tt[:], src)
                    diff = w_pool.tile([125, 2400], f32, tag="diff")
                    nc.gpsimd.tensor_tensor(
                        diff[:], tp[:], tt[:], mybir.AluOpType.subtract
                    )
                    nc.scalar.activation(diff[:], diff[:], Act.Square)
                    sv = w_pool.tile([125, 800], f32, tag="sv")
                    nc.vector.tensor_reduce(
                        sv[:],
                        diff[:].rearrange("p (v d) -> p v d", d=3),
                        axis=mybir.AxisListType.X,
                        op=mybir.AluOpType.add,
                    )
                    nc.scalar.activation(sv[:], sv[:], Act.Sqrt)
                    c0 = (b * 2 + h) * J2
                    nc.vector.tensor_reduce(
                        eb[:, c0 : c0 + J2],
                        sv[:].rearrange("p (j v) -> p j v", v=bs),
                        axis=mybir.AxisListType.X,
                        op=mybir.AluOpType.add,
                    )
            nc.sync.dma_start(out_t, eb[:])

    nc.compile()
    return nc


def _fast_host_assemble(core_outs, bnd0, bs):
    """core_outs: per-core [125, BPC*2*J2] block sums -> piece_sums [B, G+1]."""
    J2 = 800 // bs
    nblk = 125 * 2 * J2
    edges = [0] + [int(bnd0[j]) // bs for j in range(1, G + 1)] + [nblk]
    piece_sums = np.zeros((B, G + 1), dtype=np.float64)
    for c, out in enumerate(core_outs):
        out = out.reshape(125, BPC, 2 * J2)
        for bl in range(BPC):
            flat = out[:, bl, :].reshape(-1)  # g = p*(2*J2) + h*J2 + j
            csum = np.concatenate([[0.0], np.cumsum(flat, dtype=np.float64)])
            for r in range(G + 1):
                piece_sums[c * BPC + bl, r] = csum[edges[r + 1]] - csum[edges[r]]
    return piece_sums


# ---------------------------------------------------------------- device build


def _build_program(nb, supertiles, ncols, num_devices):
    import concourse.bacc as bacc
    import concourse.mybir as mybir
    from concourse.tile import TileContext

    f32 = mybir.dt.float32
    Act = mybir.ActivationFunctionType

    nc = bacc.Bacc(
        "TRN2", target_bir_lowering=False, debug=False, num_devices=num_devices
    )
    pred_t = nc.dram_tensor("pred", [nb * EPB], f32, kind="ExternalInput").ap()
    targ_t = nc.dram_tensor("target", [nb * EPB], f32, kind="ExternalInput").ap()
    out_t = nc.dram_tensor("out", [1, ncols], f32, kind="ExternalOutput").ap()

    with TileContext(nc) as tc:
        with (
            tc.tile_pool(name="io", bufs=2) as io_pool,
            tc.tile_pool(name="work", bufs=2) as w_pool,
            tc.tile_pool(name="stat", bufs=1) as s_pool,
            tc.tile_pool(name="psum", bufs=1, space="PSUM") as p_pool,
        ):
            acc = s_pool.tile([128, ncols], f32)
            ones = s_pool.tile([128, 1], f32)
            nc.gpsimd.memset(acc[:], 0.0)
            nc.gpsimd.memset(ones[:], 1.0)

            for (P, plist) in supertiles:
                ftot = sum(F for (_, F, _) in plist)
                vtot = ftot // 3
                tp = io_pool.tile([P, ftot], f32, tag="tp")
                tt = io_pool.tile([P, ftot], f32, tag="tt")
                fo = 0
                for (eoff, F, _) in plist:
                    src = pred_t[eoff : eoff + P * F].rearrange("(p f) -> p f", p=P)
                    nc.sync.dma_start(tp[:, fo : fo + F], src)
                    src = targ_t[eoff : eoff + P * F].rearrange("(p f) -> p f", p=P)
                    nc.sync.dma_start(tt[:, fo : fo + F], src)
                    fo += F
                diff = w_pool.tile([P, ftot], f32, tag="diff")
                nc.gpsimd.tensor_tensor(
                    diff[:], tp[:], tt[:], mybir.AluOpType.subtract
                )
                nc.scalar.activation(diff[:], diff[:], Act.Square)
                sv = w_pool.tile([P, vtot], f32, tag="sv")
                nc.vector.tensor_reduce(
                    sv[:],
                    diff[:].rearrange("p (v d) -> p v d", d=3),
                    axis=mybir.AxisListType.X,
                    op=mybir.AluOpType.add,
                )
                vo = 0
                for (_, F, col) in plist:
                    fv = F // 3
                    nc.scalar.activation(
                        sv[:, vo : vo + fv],
                        sv[:, vo : vo + fv],
                        Act.Sqrt,
                        accum_out=acc[:P, col : col + 1],
                    )
                    vo += fv

            outs = s_pool.tile([1, ncols], f32)
            for c0 in range(0, ncols, 512):
                c1 = min(ncols, c0 + 512)
                ps = p_pool.tile([1, c1 - c0], f32, tag="ps")
                nc.tensor.matmul(
                    ps[:], ones[:], acc[:, c0:c1], start=True, stop=True
                )
                nc.vector.tensor_copy(outs[:, c0:c1], ps[:])
            nc.sync.dma_start(out_t, outs[:])

    nc.compile()
    return nc


def _get_program(nb, bnds_key, supertiles, ncols, num_devices):
    key = (nb, bnds_key, num_devices)
    if key not in _prog_cache:
        _prog_cache[key] = _build_program(nb, supertiles, ncols, num_devices)
    return _prog_cache[key]


# ---------------------------------------------------------------- entry point

TRACE = False
LAST_RESULTS = None


def kernel(pred, target, indices, indices_type):
    global LAST_RESULTS
    from concourse.bass_utils import run_bass_kernel_spmd

    pred = np.asarray(pred, dtype=np.float32)
    target = np.asarray(target, dtype=np.float32)
    bnds = np.asarray(indices).astype(np.int64)
    itype = np.asarray(indices_type, dtype=np.float32)

    if _fast3_ok(bnds):
        key = ("fast3",)
        if key not in _prog_cache:
            _prog_cache[key] = _build_program_fast3()
        nc = _prog_cache[key]
        in_maps = [
            {
                "pred": np.ascontiguousarray(
                    pred[c * BPC : (c + 1) * BPC]
                ).reshape(-1),
                "target": np.ascontiguousarray(
                    target[c * BPC : (c + 1) * BPC]
                ).reshape(-1),
            }
            for c in range(NCORES)
        ]
        res = run_bass_kernel_spmd(nc, in_maps, list(range(NCORES)), trace=TRACE)
        LAST_RESULTS = res
        core_outs = [np.asarray(res.results[c]["out"]) for c in range(NCORES)]
        piece_sums = _fast3_host_assemble(core_outs, bnds[0])
        return _host_finish(piece_sums, bnds, itype)

    bs2 = _fast2_bs(bnds)
    if bs2 is not None:
        key = ("fast2", bs2)
        if key not in _prog_cache:
            _prog_cache[key] = _build_program_fast2(bs2)
        nc = _prog_cache[key]
        in_maps = [
            {
                "pred": np.ascontiguousarray(
                    pred[c * BPC : (c + 1) * BPC]
                ).reshape(-1),
                "target": np.ascontiguousarray(
                    target[c * BPC : (c + 1) * BPC]
                ).reshape(-1),
            }
            for c in range(NCORES)
        ]
        res = run_bass_kernel_spmd(nc, in_maps, list(range(NCORES)), trace=TRACE)
        LAST_RESULTS = res
        core_outs = [np.asarray(res.results[c]["out"]) for c in range(NCORES)]
        piece_sums = _fast2_host_assemble(core_outs, bnds[0], bs2)
        return _host_finish(piece_sums, bnds, itype)

    bs = _fast_bs(bnds)
    if bs is not None:
        key = ("fast", bs)
        if key not in _prog_cache:
            _prog_cache[key] = _build_program_fast(bs)
        nc = _prog_cache[key]
        in_maps = [
            {
                "pred": np.ascontiguousarray(
                    pred[c * BPC : (c + 1) * BPC]
                ).reshape(-1),
                "target": np.ascontiguousarray(
                    target[c * BPC : (c + 1) * BPC]
                ).reshape(-1),
            }
            for c in range(NCORES)
        ]
        res = run_bass_kernel_spmd(nc, in_maps, list(range(NCORES)), trace=TRACE)
        LAST_RESULTS = res
        core_outs = [np.asarray(res.results[c]["out"]) for c in range(NCORES)]
        piece_sums = _fast_host_assemble(core_outs, bnds[0], bs)
        return _host_finish(piece_sums, bnds, itype)

    tables = [_build_table(bnds[c * BPC : (c + 1) * BPC]) for c in range(NCORES)]
    uniform = all(t == tables[0] for t in tables[1:])

    if uniform:
        supertiles, col_map, ncols = tables[0]
        nc = _get_program(
            BPC, tuple(bnds[:BPC].ravel().tolist()), supertiles, ncols, NCORES
        )
        in_maps = [
            {
                "pred": np.ascontiguousarray(
                    pred[c * BPC : (c + 1) * BPC]
                ).reshape(-1),
                "target": np.ascontiguousarray(
                    target[c * BPC : (c + 1) * BPC]
                ).reshape(-1),
            }
            for c in range(NCORES)
        ]
        res = run_bass_kernel_spmd(
            nc, in_maps, list(range(NCORES)), trace=TRACE
        )
        LAST_RESULTS = res
        core_outs = [np.asarray(res.results[c]["out"]).ravel() for c in range(NCORES)]
        piece_sums = np.zeros((B, G + 1), dtype=np.float64)
        for c in range(NCORES):
            for col, (bl, r) in enumerate(col_map):
                piece_sums[c * BPC + bl, r] += float(core_outs[c][col])
    else:
        supertiles, col_map, ncols = _build_table(bnds)
        nc = _get_program(B, tuple(bnds.ravel().tolist()), supertiles, ncols, 1)
        in_maps = [{"pred": pred.reshape(-1), "target": target.reshape(-1)}]
        res = run_bass_kernel_spmd(nc, in_maps, [0], trace=TRACE)
        LAST_RESULTS = res
        out0 = np.asarray(res.results[0]["out"]).ravel()
        piece_sums = np.zeros((B, G + 1), dtype=np.float64)
        for col, (bl, r) in enumerate(col_map):
            piece_sums[bl, r] += float(out0[col])

    return _host_finish(piece_sums, bnds, itype)


def _host_finish(piece_sums, bnds, itype):
    # ---- host: ragged segment means + per-type means (reference semantics)
    seg_sum = np.zeros(B * G, dtype=np.float64)
    for b in range(B):
        for s in range(G):
            seg_sum[b * G + s] += piece_sums[b, s]
        fid = (b + 1) * G  # tail [bnd[8], N): sid == 8 aliases to flat (b+1)*G
        if fid < B * G:
            seg_sum[fid] += piece_sums[b, G]

    counts = (bnds[:, 1:] - bnds[:, :-1]).reshape(-1).astype(np.float64)
    with np.errstate(divide="ignore", invalid="ignore"):
        seg_mean = seg_sum / counts

    type_id = np.argmax(itype, axis=-1).reshape(-1)
    t_sum = np.zeros(T, dtype=np.float64)
    t_cnt = np.zeros(T, dtype=np.float64)
    for i in range(B * G):
        t_sum[type_id[i]] += seg_mean[i]
        t_cnt[type_id[i]] += 1.0
    with np.errstate(divide="ignore", invalid="ignore"):
        out = np.where(t_cnt > 0, t_sum / np.maximum(t_cnt, 1.0), 0.0)
    return out.astype(np.float32)



# revision 12
# speedup vs baseline: 1.3056x; 1.3056x over previous
"""Trainium2 Bass kernel for nn_L2Accuracy (segment_reduce).

Computes, for pred/target [B=32, N=200000, D=3] and ragged segment
boundaries `indices` [B, 9]:
    err[b, n] = ||pred[b,n] - target[b,n]||_2
    per-(batch, segment) sums of err  (device, 8 NeuronCores)
    segment means + per-type means    (host, O(B*G) scalars)

Data-parallel over batch, 4 batches/core.  Three device paths, fastest
applicable wins (see each builder's docstring):
  fast3   (~67-76 us): SWDGE f32->bf16-cast loads spraying all 16 SDMA
          engines, one [128, 18750] tile pair, staged column DMAs with
          pipelined bf16 compute, HWDGE f32 tail slice.  Needs equal
          per-batch boundaries, all multiples of 6250.
  fast2   (~99 us): same SWDGE-cast loads, 2 two-batch supertiles,
          unchunked compute.  Needs equal boundaries, gcd(3125,.)>=125.
  fast    (~171 us): HWDGE f32 loads, per-batch half tiles + block sums.
          Needs equal boundaries, gcd(800,.)>=50.
  generic: arbitrary boundaries, range-piece supertiles, 1 core.
Host folds device partial sums -> segment sums (incl. the reference's
tail-aliasing into the next batch's segment 0) -> segment means ->
per-type means in float64.
"""

import os
import sys

sys.path.insert(0, "/opt/trn_rl_repo")

import numpy as np

B, N, D = 32, 200000, 3
G, T = 8, 5
NCORES = 8
BPC = B // NCORES          # batches per core (fast path)
EPB = N * D                # elements per batch
FMAX_V = 1600              # max vertices per partition in one piece
STILE_V = 1600             # max vertices per partition in one supertile

_prog_cache = {}


# ---------------------------------------------------------------- host schedule


def _ranges_from_bnd(bnd):
    """9 contiguous vertex ranges partitioning [0, N) for one batch.

    Range r in 0..7 holds vertices with sid == r per the reference's
    searchsorted(bnd[1:], pos, 'right'); range 8 is the tail [bnd[8], N)
    whose vertices alias into the next batch's segment 0.
    """
    starts = [0] + [int(bnd[j]) for j in range(1, G + 1)]
    ends = [int(bnd[j]) for j in range(1, G + 1)] + [N]
    return [(s, max(0, e - s)) for s, e in zip(starts, ends)]


def _pieces_for_range(voff, vcnt):
    """Cover vcnt vertices from voff with [P, F] rects, 3 | F, P <= 128."""
    if vcnt == 0:
        return []
    for P in range(128, 63, -1):
        if vcnt % P == 0 and vcnt // P <= FMAX_V:
            return [(voff, P, 3 * (vcnt // P))]
    pieces, v, left = [], voff, vcnt
    while left > 0:
        P = min(128, left)
        fv = max(1, min(FMAX_V, left // P))
        pieces.append((v, P, 3 * fv))
        v += P * fv
        left -= P * fv
    return pieces


def _build_table(bnds):
    """Piece/supertile schedule for a list of per-batch boundary rows.

    Returns (supertiles, col_map, ncols):
      supertiles: list of (P, [(elem_off, F, col), ...])
      col_map:    col -> (batch_local, range_idx)
    """
    pieces = []          # (batch_local, range_idx, elem_off, P, F)
    for bl, bnd in enumerate(bnds):
        for r, (vs, vc) in enumerate(_ranges_from_bnd(bnd)):
            for (v0, P, F) in _pieces_for_range(vs, vc):
                pieces.append((bl, r, bl * EPB + 3 * v0, P, F))

    supertiles, col_map = [], []
    cur_p, cur_list, cur_fv = None, [], 0
    for (bl, r, eoff, P, F) in pieces:
        col = len(col_map)
        col_map.append((bl, r))
        if cur_p != P or cur_fv + F // 3 > STILE_V:
            if cur_list:
                supertiles.append((cur_p, cur_list))
            cur_p, cur_list, cur_fv = P, [], 0
        cur_list.append((eoff, F, col))
        cur_fv += F // 3
    if cur_list:
        supertiles.append((cur_p, cur_list))
    return supertiles, col_map, len(col_map)


# ---------------------------------------------------- fastest SWDGE-cast path
#
# HW findings (perfetto, axon trn2): HWDGE (nc.sync/nc.scalar) queues are
# served by only 5 SDMA engines (~134 GB/s); SWDGE (nc.gpsimd) sprays
# descriptors across all 16 engines.  With all 16 engines the SBUF-write
# port mux (~13 GB/s/engine) binds for f32, so casting f32->bf16 during
# the SWDGE DMA halves the write side and lets the HBM read side run at
# ~26.6 GB/s/engine (~430 GB/s aggregate).  bf16 compute, f32 accum.


def _fast3_ok(bnds):
    """fast3 needs equal per-batch boundaries, all multiples of 6250."""
    if not all((bnds[i] == bnds[0]).all() for i in range(1, len(bnds))):
        return False
    return all(int(v) % 6250 == 0 for v in bnds[0].tolist())


def _build_program_fast3():
    """One [128, 18750] bf16 tile pair per core (4 batches, row = 6250
    vertices, batch = 32 rows).

    Loads are SWDGE (nc.gpsimd) f32->bf16-cast DMAs — SWDGE sprays
    descriptors over all 16 SDMA engines (HWDGE queues only get 5), and
    the bf16 cast halves the SBUF-port write so each engine reads HBM at
    its ~27 GB/s line rate (~430 GB/s/core aggregate).  The loads are
    split into decreasing column stages, pred/target interleaved, so the
    per-chunk compute chain (DVE bf16 subtract -> ACT square -> two DVE
    strided adds over D=3 -> ACT sqrt with accum_out) pipelines behind
    the DMA stream and only the small last stage trails it.  The final
    HW columns go through the independent HWDGE path (nc.sync, f32,
    engines E64-68) so the tail does not wait on the slowest SWDGE
    engine (E79/DMA_15 runs ~20% slow on some runs).  eb[:, c] collects
    per-(row, chunk) err sums; the host sums chunks per row, then
    assembles 6250-vertex row blocks into segments."""
    import concourse.bacc as bacc
    import concourse.mybir as mybir
    from concourse.tile import TileContext

    f32 = mybir.dt.float32
    bf16 = mybir.dt.bfloat16
    Act = mybir.ActivationFunctionType

    W = 18750
    # SWDGE bf16 stages (col ranges, multiples of 3), decreasing so the
    # last stage (whose compute is the post-DMA tail) is small; the final
    # HW columns go via the independent HWDGE path (sync, f32, engines
    # E64-68) so the tail does not wait on the slowest SWDGE engine
    HW = 1125
    qw = [5625, 5625, 4500, 1875]
    qcols = []
    a = 0
    for w in qw:
        qcols.append((a, a + w))
        a += w
    assert a == W - HW
    # compute chunks: halves of the wide stages
    ccols = []
    for i, (a, b) in enumerate(qcols):
        if b - a > 2000:
            m = a + ((b - a) // 2 // 3) * 3
            ccols.append((a, m))
            ccols.append((m, b))
        else:
            ccols.append((a, b))
    ncc = len(ccols) + 1

    nc = bacc.Bacc(
        "TRN2", target_bir_lowering=False, debug=False, num_devices=NCORES
    )
    pred_t = nc.dram_tensor("pred", [BPC * EPB], f32, kind="ExternalInput").ap()
    targ_t = nc.dram_tensor("target", [BPC * EPB], f32, kind="ExternalInput").ap()
    out_t = nc.dram_tensor("out", [128, ncc], f32, kind="ExternalOutput").ap()

    with TileContext(nc) as tc:
        with (
            tc.tile_pool(name="io", bufs=1) as io_pool,
            tc.tile_pool(name="work", bufs=1) as w_pool,
            tc.tile_pool(name="stat", bufs=1) as s_pool,
        ):
            eb = s_pool.tile([128, ncc], f32)
            nc.gpsimd.memset(eb[:], 0.0)
            tp = io_pool.tile([128, W], bf16, tag="tp")
            tt = io_pool.tile([128, W], bf16, tag="tt")
            diff = w_pool.tile([128, W], bf16, tag="diff")
            t1 = w_pool.tile([128, W // 3], bf16, tag="t1")
            sv = w_pool.tile([128, W // 3], bf16, tag="sv")

            tpf = w_pool.tile([128, HW], f32, tag="tpf")
            ttf = w_pool.tile([128, HW], f32, tag="ttf")
            svf = w_pool.tile([128, HW // 3], f32, tag="svf")

            psrc = pred_t.rearrange("(p f) -> p f", p=128)
            tsrc = targ_t.rearrange("(p f) -> p f", p=128)
            nc.sync.dma_start(tpf[:], psrc[:, W - HW : W])
            nc.sync.dma_start(ttf[:], tsrc[:, W - HW : W])
            for (a, b) in qcols:
                nc.gpsimd.dma_start(tp[:, a:b], psrc[:, a:b])
                nc.gpsimd.dma_start(tt[:, a:b], tsrc[:, a:b])

            for ci, (a, b) in enumerate(ccols):
                nc.vector.tensor_tensor(
                    diff[:, a:b],
                    tp[:, a:b],
                    tt[:, a:b],
                    mybir.AluOpType.subtract,
                )
                nc.scalar.activation(diff[:, a:b], diff[:, a:b], Act.Square)
                va, vb = a // 3, b // 3
                sq3 = diff[:, a:b].rearrange("p (v d) -> p v d", d=3)
                nc.vector.tensor_tensor(
                    t1[:, va:vb], sq3[:, :, 0], sq3[:, :, 1], mybir.AluOpType.add
                )
                nc.vector.tensor_tensor(
                    sv[:, va:vb], t1[:, va:vb], sq3[:, :, 2], mybir.AluOpType.add
                )
                nc.scalar.activation(
                    sv[:, va:vb],
                    sv[:, va:vb],
                    Act.Sqrt,
                    accum_out=eb[:, ci : ci + 1],
                )

            # HWDGE f32 tail slice
            nc.vector.tensor_tensor(
                tpf[:], tpf[:], ttf[:], mybir.AluOpType.subtract
            )
            nc.scalar.activation(tpf[:], tpf[:], Act.Square)
            sq3f = tpf[:].rearrange("p (v d) -> p v d", d=3)
            nc.vector.tensor_tensor(
                svf[:], sq3f[:, :, 0], sq3f[:, :, 1], mybir.AluOpType.add
            )
            nc.vector.tensor_tensor(
                svf[:], svf[:], sq3f[:, :, 2], mybir.AluOpType.add
            )
            nc.scalar.activation(
                svf[:], svf[:], Act.Sqrt, accum_out=eb[:, ncc - 1 : ncc]
            )
            nc.sync.dma_start(out_t, eb[:])

    nc.compile()
    return nc


def _fast3_host_assemble(core_outs, bnd0):
    """core_outs: per-core [128, 8] chunk sums (one col per compute
    chunk) -> piece_sums [B, G+1].

    Row p of a core holds vertices [p*6250, (p+1)*6250) of the core's
    4-batch blob; batch bl = rows 32*bl .. +32."""
    piece_sums = np.zeros((B, G + 1), dtype=np.float64)
    edges = [0] + [int(bnd0[j]) // 6250 for j in range(1, G + 1)] + [32]
    for c, out in enumerate(core_outs):
        rows = out.sum(axis=1, dtype=np.float64).reshape(128)
        for bl in range(BPC):
            flat = rows[32 * bl : 32 * bl + 32]
            csum = np.concatenate([[0.0], np.cumsum(flat, dtype=np.float64)])
            for r in range(G + 1):
                piece_sums[c * BPC + bl, r] = csum[edges[r + 1]] - csum[edges[r]]
    return piece_sums


def _fast2_bs(bnds):
    import math

    if not all((bnds[i] == bnds[0]).all() for i in range(1, len(bnds))):
        return None
    bs = 3125
    for v in bnds[0].tolist():
        bs = math.gcd(bs, int(v))
    return bs if bs >= 125 else None


def _build_program_fast2(bs):
    import concourse.bacc as bacc
    import concourse.mybir as mybir
    from concourse.tile import TileContext

    f32 = mybir.dt.float32
    bf16 = mybir.dt.bfloat16
    Act = mybir.ActivationFunctionType
    J = 3125 // bs
    ncols = 2 * J

    nc = bacc.Bacc(
        "TRN2", target_bir_lowering=False, debug=False, num_devices=NCORES
    )
    pred_t = nc.dram_tensor("pred", [BPC * EPB], f32, kind="ExternalInput").ap()
    targ_t = nc.dram_tensor("target", [BPC * EPB], f32, kind="ExternalInput").ap()
    out_t = nc.dram_tensor("out", [128, ncols], f32, kind="ExternalOutput").ap()

    with TileContext(nc) as tc:
        with (
            tc.tile_pool(name="io", bufs=4) as io_pool,
            tc.tile_pool(name="work", bufs=2) as w_pool,
            tc.tile_pool(name="stat", bufs=1) as s_pool,
        ):
            eb = s_pool.tile([128, ncols], f32)
            nc.gpsimd.memset(eb[:], 0.0)
            for s in range(2):
                # supertile s = batches (2s, 2s+1): elems [2s*EPB, (2s+2)*EPB)
                tp = io_pool.tile([128, 9375], bf16, tag="tp")
                tt = io_pool.tile([128, 9375], bf16, tag="tt")
                src = pred_t[2 * s * EPB : (2 * s + 2) * EPB].rearrange(
                    "(p f) -> p f", p=128
                )
                nc.gpsimd.dma_start(tp[:], src)
                src = targ_t[2 * s * EPB : (2 * s + 2) * EPB].rearrange(
                    "(p f) -> p f", p=128
                )
                nc.gpsimd.dma_start(tt[:], src)
                diff = w_pool.tile([128, 9375], bf16, tag="diff")
                nc.vector.tensor_tensor(
                    diff[:], tp[:], tt[:], mybir.AluOpType.subtract
                )
                nc.scalar.activation(diff[:], diff[:], Act.Square)
                sv = w_pool.tile([128, 3125], f32, tag="sv")
                nc.vector.tensor_reduce(
                    sv[:],
                    diff[:].rearrange("p (v d) -> p v d", d=3),
                    axis=mybir.AxisListType.X,
                    op=mybir.AluOpType.add,
                )
                if J == 1:
                    nc.scalar.activation(
                        sv[:], sv[:], Act.Sqrt, accum_out=eb[:, s : s + 1]
                    )
                else:
                    nc.scalar.activation(sv[:], sv[:], Act.Sqrt)
                    nc.vector.tensor_reduce(
                        eb[:, s * J : (s + 1) * J],
                        sv[:].rearrange("p (j v) -> p j v", v=bs),
                        axis=mybir.AxisListType.X,
                        op=mybir.AluOpType.add,
                    )
            nc.sync.dma_start(out_t, eb[:])

    nc.compile()
    return nc


def _fast2_host_assemble(core_outs, bnd0, bs):
    """core_outs: per-core [128, 2J] block sums -> piece_sums [B, G+1].

    Batch bl of a core lives in supertile s = bl//2, rows 64*(bl%2)..+64;
    within a batch the flat block order is (row, j), block g covering
    vertices [g*bs, (g+1)*bs).
    """
    J = 3125 // bs
    nblk = 64 * J
    edges = [0] + [int(bnd0[j]) // bs for j in range(1, G + 1)] + [nblk]
    piece_sums = np.zeros((B, G + 1), dtype=np.float64)
    for c, out in enumerate(core_outs):
        for bl in range(BPC):
            s, half = divmod(bl, 2)
            flat = out[64 * half : 64 * half + 64, s * J : (s + 1) * J].reshape(-1)
            csum = np.concatenate([[0.0], np.cumsum(flat, dtype=np.float64)])
            for r in range(G + 1):
                piece_sums[c * BPC + bl, r] = csum[edges[r + 1]] - csum[edges[r]]
    return piece_sums


# ------------------------------------------------------- fast block-sum path
#
# When all batches share one boundary vector whose entries divide by a
# block size bs (bs | 800, bs >= 50), each batch is two [125, 2400]-elem
# half-tiles (one contiguous 9.6 KB run per partition -> ~125 DMA packets
# per 1.2 MB DMA instead of per-range shattering), and per-(row, block)
# err sums [125, 2*J2] per batch stream out for host reduceat assembly.


def _fast_bs(bnds):
    import math

    if not all((bnds[i] == bnds[0]).all() for i in range(1, len(bnds))):
        return None
    bs = 800
    for v in bnds[0].tolist():
        bs = math.gcd(bs, int(v))
    return bs if bs >= 50 else None


def _build_program_fast(bs):
    import concourse.bacc as bacc
    import concourse.mybir as mybir
    from concourse.tile import TileContext

    f32 = mybir.dt.float32
    Act = mybir.ActivationFunctionType
    J2 = 800 // bs  # blocks per half-row
    ncols = BPC * 2 * J2

    nc = bacc.Bacc(
        "TRN2", target_bir_lowering=False, debug=False, num_devices=NCORES
    )
    pred_t = nc.dram_tensor("pred", [BPC * EPB], f32, kind="ExternalInput").ap()
    targ_t = nc.dram_tensor("target", [BPC * EPB], f32, kind="ExternalInput").ap()
    out_t = nc.dram_tensor("out", [125, ncols], f32, kind="ExternalOutput").ap()

    with TileContext(nc) as tc:
        with (
            tc.tile_pool(name="io", bufs=4) as io_pool,
            tc.tile_pool(name="work", bufs=3) as w_pool,
            tc.tile_pool(name="stat", bufs=1) as s_pool,
        ):
            eb = s_pool.tile([125, ncols], f32)
            for b in range(BPC):
                for h in range(2):
                    # partition p holds elements [b*EPB + 4800p + 2400h, +2400)
                    tp = io_pool.tile([125, 2400], f32, tag="tp")
                    tt = io_pool.tile([125, 2400], f32, tag="tt")
                    src = pred_t[b * EPB : (b + 1) * EPB].rearrange(
                        "(p f) -> p f", p=125
                    )[:, 2400 * h : 2400 * h + 2400]
                    nc.sync.dma_start(tp[:], src)
                    src = targ_t[b * EPB : (b + 1) * EPB].rearrange(
                        "(p f) -> p f", p=125
                    )[:, 2400 * h : 2400 * h + 2400]
                    nc.sync.dma_start(tt[:], src)
                    diff = w_pool.tile([125, 2400], f32, tag="diff")
                    nc.gpsimd.tensor_tensor(
                        diff[:], tp[:], tt[:], mybir.AluOpType.subtract
                    )
                    nc.scalar.activation(diff[:], diff[:], Act.Square)
                    sv = w_pool.tile([125, 800], f32, tag="sv")
                    nc.vector.tensor_reduce(
                        sv[:],
                        diff[:].rearrange("p (v d) -> p v d", d=3),
                        axis=mybir.AxisListType.X,
                        op=mybir.AluOpType.add,
                    )
                    nc.scalar.activation(sv[:], sv[:], Act.Sqrt)
                    c0 = (b * 2 + h) * J2
                    nc.vector.tensor_reduce(
                        eb[:, c0 : c0 + J2],
                        sv[:].rearrange("p (j v) -> p j v", v=bs),
                        axis=mybir.AxisListType.X,
                        op=mybir.AluOpType.add,
                    )
            nc.sync.dma_start(out_t, eb[:])

    nc.compile()
    return nc


def _fast_host_assemble(core_outs, bnd0, bs):
    """core_outs: per-core [125, BPC*2*J2] block sums -> piece_sums [B, G+1]."""
    J2 = 800 // bs
    nblk = 125 * 2 * J2
    edges = [0] + [int(bnd0[j]) // bs for j in range(1, G + 1)] + [nblk]
    piece_sums = np.zeros((B, G + 1), dtype=np.float64)
    for c, out in enumerate(core_outs):
        out = out.reshape(125, BPC, 2 * J2)
        for bl in range(BPC):
            flat = out[:, bl, :].reshape(-1)  # g = p*(2*J2) + h*J2 + j
            csum = np.concatenate([[0.0], np.cumsum(flat, dtype=np.float64)])
            for r in range(G + 1):
                piece_sums[c * BPC + bl, r] = csum[edges[r + 1]] - csum[edges[r]]
    return piece_sums


# ---------------------------------------------------------------- device build


def _build_program(nb, supertiles, ncols, num_devices):
    import concourse.bacc as bacc
    import concourse.mybir as mybir
    from concourse.tile import TileContext

    f32 = mybir.dt.float32
    Act = mybir.ActivationFunctionType

    nc = bacc.Bacc(
        "TRN2", target_bir_lowering=False, debug=False, num_devices=num_devices
    )
    pred_t = nc.dram_tensor("pred", [nb * EPB], f32, kind="ExternalInput").ap()
    targ_t = nc.dram_tensor("target", [nb * EPB], f32, kind="ExternalInput").ap()
    out_t = nc.dram_tensor("out", [1, ncols], f32, kind="ExternalOutput").ap()

    with TileContext(nc) as tc:
        with (
            tc.tile_pool(name="io", bufs=2) as io_pool,
            tc.tile_pool(name="work", bufs=2) as w_pool,
            tc.tile_pool(name="stat", bufs=1) as s_pool,
            tc.tile_pool(name="psum", bufs=1, space="PSUM") as p_pool,
        ):
            acc = s_pool.tile([128, ncols], f32)
            ones = s_pool.tile([128, 1], f32)
            nc.gpsimd.memset(acc[:], 0.0)
            nc.gpsimd.memset(ones[:], 1.0)

            for (P, plist) in supertiles:
                ftot = sum(F for (_, F, _) in plist)
                vtot = ftot // 3
                tp = io_pool.tile([P, ftot], f32, tag="tp")
                tt = io_pool.tile([P, ftot], f32, tag="tt")
                fo = 0
                for (eoff, F, _) in plist:
                    src = pred_t[eoff : eoff + P * F].rearrange("(p f) -> p f", p=P)
                    nc.sync.dma_start(tp[:, fo : fo + F], src)
                    src = targ_t[eoff : eoff + P * F].rearrange("(p f) -> p f", p=P)
                    nc.sync.dma_start(tt[:, fo : fo + F], src)
                    fo += F
                diff = w_pool.tile([P, ftot], f32, tag="diff")
                nc.gpsimd.tensor_tensor(
                    diff[:], tp[:], tt[:], mybir.AluOpType.subtract
                )
                nc.scalar.activation(diff[:], diff[:], Act.Square)
                sv = w_pool.tile([P, vtot], f32, tag="sv")
                nc.vector.tensor_reduce(
                    sv[:],
                    diff[:].rearrange("p (v d) -> p v d", d=3),
                    axis=mybir.AxisListType.X,
                    op=mybir.AluOpType.add,
                )
                vo = 0
                for (_, F, col) in plist:
                    fv = F // 3
                    nc.scalar.activation(
                        sv[:, vo : vo + fv],
                        sv[:, vo : vo + fv],
                        Act.Sqrt,
                        accum_out=acc[:P, col : col + 1],
                    )
                    vo += fv

            outs = s_pool.tile([1, ncols], f32)
            for c0 in range(0, ncols, 512):
                c1 = min(ncols, c0 + 512)
                ps = p_pool.tile([1, c1 - c0], f32, tag="ps")
                nc.tensor.matmul(
                    ps[:], ones[:], acc[:, c0:c1], start=True, stop=True
                )
                nc.vector.tensor_copy(outs[:, c0:c1], ps[:])
            nc.sync.dma_start(out_t, outs[:])

    nc.compile()
    return nc


def _get_program(nb, bnds_key, supertiles, ncols, num_devices):
    key = (nb, bnds_key, num_devices)
    if key not in _prog_cache:
        _prog_cache[key] = _build_program(nb, supertiles, ncols, num_devices)
    return _prog_cache[key]


# ---------------------------------------------------------------- entry point

TRACE = False
LAST_RESULTS = None


def kernel(pred, target, indices, indices_type):
    global LAST_RESULTS
    from concourse.bass_utils import run_bass_kernel_spmd

    pred = np.asarray(pred, dtype=np.float32)
    target = np.asarray(target, dtype=np.float32)
    bnds = np.asarray(indices).astype(np.int64)
    itype = np.asarray(indices_type, dtype=np.float32)

    if _fast3_ok(bnds):
        key = ("fast3",)
        if key not in _prog_cache:
            _prog_cache[key] = _build_program_fast3()
        nc = _prog_cache[key]
        in_maps = [
            {
                "pred": np.ascontiguousarray(
                    pred[c * BPC : (c + 1) * BPC]
                ).reshape(-1),
                "target": np.ascontiguousarray(
                    target[c * BPC : (c + 1) * BPC]
                ).reshape(-1),
            }
            for c in range(NCORES)
        ]
        res = run_bass_kernel_spmd(nc, in_maps, list(range(NCORES)), trace=TRACE)
        LAST_RESULTS = res
        core_outs = [np.asarray(res.results[c]["out"]) for c in range(NCORES)]
        piece_sums = _fast3_host_assemble(core_outs, bnds[0])
        return _host_finish(piece_sums, bnds, itype)

    bs2 = _fast2_bs(bnds)
    if bs2 is not None:
        key = ("fast2", bs2)
        if key not in _prog_cache:
            _prog_cache[key] = _build_program_fast2(bs2)
        nc = _prog_cache[key]
        in_maps = [
            {
                "pred": np.ascontiguousarray(
                    pred[c * BPC : (c + 1) * BPC]
                ).reshape(-1),
                "target": np.ascontiguousarray(
                    target[c * BPC : (c + 1) * BPC]
                ).reshape(-1),
            }
            for c in range(NCORES)
        ]
        res = run_bass_kernel_spmd(nc, in_maps, list(range(NCORES)), trace=TRACE)
        LAST_RESULTS = res
        core_outs = [np.asarray(res.results[c]["out"]) for c in range(NCORES)]
        piece_sums = _fast2_host_assemble(core_outs, bnds[0], bs2)
        return _host_finish(piece_sums, bnds, itype)

    bs = _fast_bs(bnds)
    if bs is not None:
        key = ("fast", bs)
        if key not in _prog_cache:
            _prog_cache[key] = _build_program_fast(bs)
        nc = _prog_cache[key]
        in_maps = [
            {
                "pred": np.ascontiguousarray(
                    pred[c * BPC : (c + 1) * BPC]
                ).reshape(-1),
                "target": np.ascontiguousarray(
                    target[c * BPC : (c + 1) * BPC]
                ).reshape(-1),
            }
            for c in range(NCORES)
        ]
        res = run_bass_kernel_spmd(nc, in_maps, list(range(NCORES)), trace=TRACE)
        LAST_RESULTS = res
        core_outs = [np.asarray(res.results[c]["out"]) for c in range(NCORES)]
        piece_sums = _fast_host_assemble(core_outs, bnds[0], bs)
        return _host_finish(piece_sums, bnds, itype)

    tables = [_build_table(bnds[c * BPC : (c + 1) * BPC]) for c in range(NCORES)]
    uniform = all(t == tables[0] for t in tables[1:])

    if uniform:
        supertiles, col_map, ncols = tables[0]
        nc = _get_program(
            BPC, tuple(bnds[:BPC].ravel().tolist()), supertiles, ncols, NCORES
        )
        in_maps = [
            {
                "pred": np.ascontiguousarray(
                    pred[c * BPC : (c + 1) * BPC]
                ).reshape(-1),
                "target": np.ascontiguousarray(
                    target[c * BPC : (c + 1) * BPC]
                ).reshape(-1),
            }
            for c in range(NCORES)
        ]
        res = run_bass_kernel_spmd(
            nc, in_maps, list(range(NCORES)), trace=TRACE
        )
        LAST_RESULTS = res
        core_outs = [np.asarray(res.results[c]["out"]).ravel() for c in range(NCORES)]
        piece_sums = np.zeros((B, G + 1), dtype=np.float64)
        for c in range(NCORES):
            for col, (bl, r) in enumerate(col_map):
                piece_sums[c * BPC + bl, r] += float(core_outs[c][col])
    else:
        supertiles, col_map, ncols = _build_table(bnds)
        nc = _get_program(B, tuple(bnds.ravel().tolist()), supertiles, ncols, 1)
        in_maps = [{"pred": pred.reshape(-1), "target": target.reshape(-1)}]
        res = run_bass_kernel_spmd(nc, in_maps, [0], trace=TRACE)
        LAST_RESULTS = res
        out0 = np.asarray(res.results[0]["out"]).ravel()
        piece_sums = np.zeros((B, G + 1), dtype=np.float64)
        for col, (bl, r) in enumerate(col_map):
            piece_sums[bl, r] += float(out0[col])

    return _host_finish(piece_sums, bnds, itype)


def _host_finish(piece_sums, bnds, itype):
    # ---- host: ragged segment means + per-type means (reference semantics)
    seg_sum = np.zeros(B * G, dtype=np.float64)
    for b in range(B):
        for s in range(G):
            seg_sum[b * G + s] += piece_sums[b, s]
        fid = (b + 1) * G  # tail [bnd[8], N): sid == 8 aliases to flat (b+1)*G
        if fid < B * G:
            seg_sum[fid] += piece_sums[b, G]

    counts = (bnds[:, 1:] - bnds[:, :-1]).reshape(-1).astype(np.float64)
    with np.errstate(divide="ignore", invalid="ignore"):
        seg_mean = seg_sum / counts

    type_id = np.argmax(itype, axis=-1).reshape(-1)
    t_sum = np.zeros(T, dtype=np.float64)
    t_cnt = np.zeros(T, dtype=np.float64)
    for i in range(B * G):
        t_sum[type_id[i]] += seg_mean[i]
        t_cnt[type_id[i]] += 1.0
    with np.errstate(divide="ignore", invalid="ignore"):
        out = np.where(t_cnt > 0, t_sum / np.maximum(t_cnt, 1.0), 0.0)
    return out.astype(np.float32)

